# revision 8
# baseline (speedup 1.0000x reference)
"""Multi-head attention (QKV proj + per-head bias + softmax + out proj) on 8 TRN2 NeuronCores.

Sharding: data-parallel over batch B=4 x tensor-parallel over head-groups
(12 heads -> 2 groups of 6). core = b*2 + hg. Each core computes its 6 heads'
full attention for one batch element plus the partial output projection over
its heads' rows of W_proj; the two partials per batch are summed on the host
(the deferred all-reduce), where b_proj is also added.

Device-side layout notes:
- Everything runs transposed (feature dim on partitions): x^T, q^T, k^T feed
  the PE directly; softmax runs on S^T tiles [m(keys) x n(queries)] so exp is
  elementwise and the denominator comes free from an extra ones-column in the
  AV matmul's stationary operand ([v | 1] -> U rows 0..63 = unnormalized out,
  row 64 = sum of exp). Normalization multiplies by 1/denom broadcast across
  partitions via gpsimd.partition_broadcast.
- Matmul inputs are float32r (FP22-truncated fp32): full PE rate at moving
  free-dim >= 256, ~1e-4 relative error.
"""

import numpy as np

import concourse.bacc as bacc
import concourse.tile as tile
import concourse.mybir as mybir
from concourse.bass_utils import run_bass_kernel_spmd

B, N, C, H, HD = 4, 2048, 768, 12, 64
HL = 6                 # heads per core
CL = HL * HD           # 384 local qkv width
SCALE = HD ** -0.5
P = 128
NB = 512               # query-block (n) size
NBS = N // NB          # 4
MC = N // P            # 16 key-chunks (m)
KC = C // P            # 6 contraction chunks of C
PAIRS = HL // 2        # 3 head pairs (stacked 64+64 on partitions)
D1 = HD + 1            # v augmented with ones column

f32 = mybir.dt.float32
f32r = mybir.dt.float32r
EXP = mybir.ActivationFunctionType.Exp

_CACHE: dict = {}


def _build():
    nc = bacc.Bacc("TRN2", target_bir_lowering=False, debug=False, num_devices=8)

    xt = nc.dram_tensor("xt", [C, N], f32r, kind="ExternalInput")        # x^T
    wq = nc.dram_tensor("wq", [C, CL], f32r, kind="ExternalInput")
    wk = nc.dram_tensor("wk", [C, CL], f32r, kind="ExternalInput")
    wv = nc.dram_tensor("wv", [C, CL], f32r, kind="ExternalInput")
    qb = nc.dram_tensor("qb", [PAIRS, P, N], f32, kind="ExternalInput")  # qbias^T + b_q
    kb = nc.dram_tensor("kb", [PAIRS, P, N], f32, kind="ExternalInput")
    vb = nc.dram_tensor("vb", [N, CL], f32, kind="ExternalInput")        # vbias + b_v
    wp = nc.dram_tensor("wp", [CL, C], f32r, kind="ExternalInput")       # W_proj local rows
    ot = nc.dram_tensor("ot", [C, N], f32, kind="ExternalOutput")        # partial out^T

    xt_r = xt.ap().rearrange("(co p) n -> p co n", p=P)
    wq_r = wq.ap().rearrange("(co p) j -> p co j", p=P)
    wk_r = wk.ap().rearrange("(co p) j -> p co j", p=P)
    wv_r = wv.ap().rearrange("(co p) j -> p co j", p=P)
    wp_r = wp.ap().rearrange("(c3 p) c -> p c3 c", p=P)
    vb_r = vb.ap().rearrange("(mc p) j -> p mc j", p=P)
    ot_r = ot.ap().rearrange("(cc p) n -> p cc n", p=P)

    with tile.TileContext(nc) as tc:
        with (
            tc.tile_pool(name="persist", bufs=1) as pp,
            tc.tile_pool(name="stream", bufs=2) as sp,
            tc.tile_pool(name="ps", bufs=2, space="PSUM") as ps,
        ):
            # ---- persistent tiles ----
            wq_sb = pp.tile([P, KC, CL], f32r)
            wk_sb = pp.tile([P, KC, CL], f32r)
            wv_sb = pp.tile([P, KC, CL], f32r)
            wp_sb = pp.tile([P, PAIRS, C], f32r)
            qT = pp.tile([P, PAIRS, N], f32r)    # q^T (pair-stacked heads)
            kT = pp.tile([P, PAIRS, N], f32r)    # k^T
            v_aug = pp.tile([P, MC, HL, D1], f32r)  # [v | 1] per m-chunk/head
            ones_f32 = pp.tile([P, 1], f32)

            nc.sync.dma_start(wq_sb[:], wq_r)
            nc.sync.dma_start(wk_sb[:], wk_r)
            nc.sync.dma_start(wv_sb[:], wv_r)
            nc.sync.dma_start(wp_sb[:], wp_r)

            # PE warmup: ~5us of dense dummy matmuls flips the HAM clock
            # gate to 8/8 (2.4 GHz) before the real work arrives.
            warm_a = pp.tile([P, P], f32r)
            warm_b = pp.tile([P, NB], f32r)
            nc.vector.memset(warm_a.bitcast(f32)[:], 0.0)
            nc.vector.memset(warm_b.bitcast(f32)[:], 0.0)
            wps = ps.tile([P, NB], f32, tag="mps", name="warm_ps")
            for _ in range(24):
                nc.tensor.matmul(wps[:], warm_a[:], warm_b[:], start=True, stop=True)

            nc.vector.memset(ones_f32[:], 1.0)
            with nc.allow_low_precision(reason="ones column is exact in f32r"):
                nc.vector.tensor_copy(
                    v_aug[:, :, :, HD], ones_f32.to_broadcast([P, MC, HL])
                )

            # ---- prologue: stream x^T in n-blocks; produce q^T, k^T, v ----
            for nb in range(NBS):
                ns = slice(nb * NB, (nb + 1) * NB)
                xt_blk = sp.tile([P, KC, NB], f32r, tag="xt", bufs=2)
                nc.sync.dma_start(xt_blk[:], xt_r[:, :, ns])

                for c3 in range(PAIRS):
                    js = slice(c3 * P, (c3 + 1) * P)
                    for (w_sb, bias_dram, dst) in ((wq_sb, qb, qT), (wk_sb, kb, kT)):
                        mm = ps.tile([P, NB], f32, tag="sps", name=f"qk_{nb}_{c3}")
                        for co in range(KC):
                            nc.tensor.matmul(
                                mm[:], w_sb[:, co, js], xt_blk[:, co, :],
                                start=(co == 0), stop=(co == KC - 1),
                            )
                        bias_t = sp.tile([P, NB], f32, tag="bias", bufs=4,
                                         name=f"b_{nb}_{c3}")
                        nc.sync.dma_start(bias_t[:], bias_dram.ap()[c3, :, ns])
                        with nc.allow_low_precision(reason="f32r matmul operand"):
                            nc.vector.tensor_add(dst[:, c3, ns], mm[:], bias_t[:])

                for ch in range(NB // P):
                    mchunk = nb * (NB // P) + ch
                    cs = slice(ch * P, (ch + 1) * P)
                    mm = ps.tile([P, NB], f32, tag="sps", name=f"v_{nb}_{ch}")
                    for co in range(KC):
                        nc.tensor.matmul(
                            mm[:, :CL], xt_blk[:, co, cs], wv_sb[:, co, :],
                            start=(co == 0), stop=(co == KC - 1),
                        )
                    vb_t = sp.tile([P, CL], f32, tag="vb", bufs=3,
                                   name=f"vb_{mchunk}")
                    nc.sync.dma_start(vb_t[:], vb_r[:, mchunk, :])
                    with nc.allow_low_precision(reason="f32r matmul operand"):
                        nc.vector.tensor_add(
                            v_aug[:, mchunk, :, 0:HD], mm[:, :CL], vb_t[:]
                        )

            # ---- attention + projection ----
            # Software-pipelined over steps (nb, c3, mc): the S matmuls for
            # step i+1 are emitted BEFORE the AV matmuls of step i, so the PE
            # (strict-FIFO queue) computes S(i+1) while ACT runs exp(i), AV(i)
            # runs during exp(i+1), and ACT stays back-to-back on exps.
            steps = [(nb, c3, mc)
                     for nb in range(NBS)
                     for c3 in range(PAIRS)
                     for mc in range(MC)]
            o_blks = {}
            u_cur = {}
            sps_tiles = {}
            exp_tiles = {}

            def emit_s(i):
                nb, c3, mc = steps[i]
                ns = slice(nb * NB, (nb + 1) * NB)
                ms = slice(mc * P, (mc + 1) * P)
                sps = ps.tile([P, 2, NB], f32, tag="sps", name=f"s_{nb}_{c3}_{mc}")
                sps_tiles[i] = sps
                for hp in range(2):
                    hb = slice(hp * HD, (hp + 1) * HD)
                    nc.tensor.matmul(
                        sps[:, hp, :], kT[hb, c3, ms], qT[hb, c3, ns],
                        start=True, stop=True,
                    )

            def emit_exp(i):
                nb, c3, mc = steps[i]
                exps = sp.tile([P, 2, NB], f32r, tag="exps", bufs=3,
                               name=f"e_{nb}_{c3}_{mc}")
                exp_tiles[i] = exps
                nc.scalar.activation(exps[:], sps_tiles.pop(i)[:], EXP, scale=SCALE)

            def emit_av(i):
                nb, c3, mc = steps[i]
                if mc == 0:
                    u_cur[0] = ps.tile([D1, NB], f32, tag="ups", name=f"u_{nb}_{c3}_0")
                    u_cur[1] = ps.tile([D1, NB], f32, tag="ups", name=f"u_{nb}_{c3}_1")
                exps = exp_tiles.pop(i)
                for hp in range(2):
                    nc.tensor.matmul(
                        u_cur[hp][:], v_aug[:, mc, c3 * 2 + hp, :],
                        exps[:, hp, :],
                        start=(mc == 0), stop=(mc == MC - 1),
                    )

            def emit_normalize(nb, c3):
                # Drain U psum fast (two cheap copies), then normalize off the
                # PSUM critical path. partition_broadcast can only write at
                # base 0, so broadcast to all 128 rows and read the 64-row
                # window matching each head's base (SB-SB ops need equal
                # input bases).
                o_blk = o_blks[nb]
                for hp in range(2):
                    u = u_cur[hp]
                    hb = slice(hp * HD, (hp + 1) * HD)
                    den = sp.tile([1, NB], f32, tag="den", bufs=4,
                                  name=f"d_{nb}_{c3}_{hp}")
                    nc.vector.tensor_copy(den[:], u[HD:D1, :])
                    with nc.allow_low_precision(reason="f32r matmul operand"):
                        nc.vector.tensor_copy(o_blk[hb, c3, :], u[0:HD, :])
                    rec = sp.tile([1, NB], f32, tag="rec", bufs=4,
                                  name=f"r_{nb}_{c3}_{hp}")
                    nc.vector.reciprocal_approx_fast(rec[:], den[:])
                    bc = sp.tile([P, NB], f32, tag="bc", bufs=3,
                                 name=f"bc_{nb}_{c3}_{hp}")
                    nc.gpsimd.partition_broadcast(bc[:], rec[:])
                    with nc.allow_low_precision(reason="f32r matmul operand"):
                        nc.vector.tensor_mul(
                            o_blk[hb, c3, :], o_blk[hb, c3, :], bc[hb, :]
                        )

            def emit_proj(nb):
                ns = slice(nb * NB, (nb + 1) * NB)
                o_blk = o_blks.pop(nb)
                for cc in range(C // P):
                    cs = slice(cc * P, (cc + 1) * P)
                    mm = ps.tile([P, NB], f32, tag="mps", name=f"p_{nb}_{cc}")
                    for c3 in range(PAIRS):
                        nc.tensor.matmul(
                            mm[:], wp_sb[:, c3, cs], o_blk[:, c3, :],
                            start=(c3 == 0), stop=(c3 == PAIRS - 1),
                        )
                    ot_t = sp.tile([P, NB], f32, tag="ot", bufs=3,
                                   name=f"ot_{nb}_{cc}")
                    nc.vector.tensor_copy(ot_t[:], mm[:])
                    nc.sync.dma_start(ot_r[:, cc, ns], ot_t[:])

            for i, (nb, c3, mc) in enumerate(steps):
                if mc == 0 and c3 == 0:
                    o_blks[nb] = sp.tile([P, PAIRS, NB], f32r, tag="oblk",
                                         bufs=2, name=f"o_{nb}")
                if i == 0:
                    emit_s(0)
                emit_exp(i)
                if i + 1 < len(steps):
                    emit_s(i + 1)
                emit_av(i)
                if mc == MC - 1:
                    emit_normalize(nb, c3)
                # Emit proj(nb-1) deep into nb's stream: by then o_blk(nb-1)
                # is long ready, so the proj matmuls never block the PE FIFO.
                if nb > 0 and c3 == 1 and mc == 4:
                    emit_proj(nb - 1)
            emit_proj(NBS - 1)

    nc.compile()
    return nc


def _get_nc():
    if "nc" not in _CACHE:
        _CACHE["nc"] = _build()
    return _CACHE["nc"]


def _prep_in_maps(x, qbias, kbias, vbias, W_qkv, b_qkv, W_proj):
    x = np.asarray(x, dtype=np.float32)
    qbias = np.asarray(qbias, dtype=np.float32)
    kbias = np.asarray(kbias, dtype=np.float32)
    vbias = np.asarray(vbias, dtype=np.float32)
    W_qkv = np.asarray(W_qkv, dtype=np.float32)
    b_qkv = np.asarray(b_qkv, dtype=np.float32)
    W_proj = np.asarray(W_proj, dtype=np.float32)

    xts = [np.ascontiguousarray(x[b].T) for b in range(B)]
    in_maps = []
    for core in range(8):
        b, hg = core // 2, core % 2
        heads = slice(hg * HL, (hg + 1) * HL)
        qcols = slice(hg * CL, (hg + 1) * CL)
        kcols = slice(C + hg * CL, C + (hg + 1) * CL)
        vcols = slice(2 * C + hg * CL, 2 * C + (hg + 1) * CL)

        # per-head bias + projection bias, transposed to [pair, 128, N]
        qb_ = qbias[b, heads] + b_qkv[qcols].reshape(HL, 1, HD)   # [6, N, 64]
        kb_ = kbias[b, heads] + b_qkv[kcols].reshape(HL, 1, HD)
        qb_t = np.ascontiguousarray(qb_.transpose(0, 2, 1)).reshape(PAIRS, P, N)
        kb_t = np.ascontiguousarray(kb_.transpose(0, 2, 1)).reshape(PAIRS, P, N)
        # v bias in natural [N, 384] (heads side by side, matching Wv columns)
        vb_ = vbias[b, heads] + b_qkv[vcols].reshape(HL, 1, HD)   # [6, N, 64]
        vb_n = np.ascontiguousarray(vb_.transpose(1, 0, 2)).reshape(N, CL)

        in_maps.append({
            "xt": xts[b],
            "wq": np.ascontiguousarray(W_qkv[:, qcols]),
            "wk": np.ascontiguousarray(W_qkv[:, kcols]),
            "wv": np.ascontiguousarray(W_qkv[:, vcols]),
            "qb": qb_t,
            "kb": kb_t,
            "vb": vb_n,
            "wp": np.ascontiguousarray(W_proj[hg * CL:(hg + 1) * CL, :]),
        })
    return in_maps


def kernel(x, qbias, kbias, vbias, W_qkv, b_qkv, W_proj, b_proj, **run_kwargs):
    nc = _get_nc()
    in_maps = _prep_in_maps(x, qbias, kbias, vbias, W_qkv, b_qkv, W_proj)
    res = run_bass_kernel_spmd(nc, in_maps, core_ids=list(range(8)), **run_kwargs)
    _CACHE["last_results"] = res

    b_proj = np.asarray(b_proj, dtype=np.float32)
    out = np.empty((B, N, C), dtype=np.float32)
    for b in range(B):
        part = res.results[2 * b]["ot"] + res.results[2 * b + 1]["ot"]  # [C, N]
        out[b] = part.T + b_proj
    return out
